# revision 15
# baseline (speedup 1.0000x reference)
"""Trainium2 Bass kernel for nn_ExtendedDecoderLayer (dense transformer decoder
layer with Shaw-style relative attention, rel-eb variant).

Sharding: sequence-parallel over 8 NeuronCores. Core c owns query rows
q in [128c, 128(c+1)) of every batch element, for all 16 heads, both attention
blocks, and the matching 512 (b, q) rows of every row-wise op. K/V projections
are computed sharded over the same rows (as key positions) and exchanged with
an 8-core AllGather.

Relative-position scores: with P = q @ rel_k.T and OH the one-hot of rel_ids
(+1 mask row), rel_score[q,k] = sum_r (P+rel_b)[q,r] * OH[q,k,r]. Two query
rows are packed per 128-deep contraction (m = 2q x 64bh), computed on the
TensorEngine against a host-built fp8 one-hot, then DMA-shuffled into
query-partition layout.

kernel(**inputs) -> np.ndarray takes FULL unsharded inputs, returns the full
[4, 1024, 1024] float32 output.
"""

import numpy as np
import ml_dtypes

B = 4
L = 1024
D = 1024
H = 16
DH = 64
FF = 4096
NREL = 32
NC = 8
QC = L // NC          # 128
RROWS = B * QC        # 512
KR = NREL + 1         # 33 (one-hot + mask row)
MASKVAL = -240.0      # max-finite e4m3; exp(-240) flushes to 0
EPS = 1e-5

KSEC = 8 * 128 * RROWS      # K section elems per core slab
VSEC = B * 128 * D          # V section
LOCN = KSEC + VSEC

_BUILT = None
DEBUG = False


def _build_program():
    import contextlib
    import concourse.bass as bass
    import concourse.mybir as mybir
    import concourse.tile as tile
    from concourse import bacc
    from concourse.masks import make_identity

    dt = mybir.dt
    AF = mybir.ActivationFunctionType
    OP = mybir.AluOpType
    BF = dt.bfloat16
    F8 = dt.float8e4
    FP = dt.float32

    nc = bacc.Bacc(None, target_bir_lowering=False)

    def din(name, shape, dtype):
        return nc.declare_dram_parameter(name, list(shape), dtype, isOutput=False)

    xT_in = din("xT", (D, RROWS), BF)
    encT_in = din("encT", (D, RROWS), BF)
    xres_in = din("xres", (RROWS, D), FP)
    w_in = {}
    for s in ("1", "2"):
        for nm in ("wqT", "wkT", "wvT", "woT"):
            w_in[nm + s] = din(nm + s, (D, D), BF)
        w_in["relkT" + s] = din("relkT" + s, (2 * DH, NREL), BF)
        w_in["rbT" + s] = din("rbT" + s, (NREL, H), FP)
        w_in["oht" + s] = din("oht" + s, (QC // 2, 128, L), F8)
    w1T_in = din("w1T", (D, FF), BF)
    w2T_in = din("w2T", (FF, D), BF)
    bcol_in = {}
    for nm in ("bq1s", "bk1s", "bq2s", "bk2s"):
        bcol_in[nm] = din(nm, (128, 8), FP)
    bcol_in["b1s"] = din("b1s", (128, FF // 128), FP)
    mrow_in = din("mrow", (1, 64 * 64), dt.float8e4)
    ln_in = {}
    for s in ("1", "2", "3"):
        ln_in["g" + s] = din("lng" + s, (1, D), FP)
        ln_in["b" + s] = din("lnb" + s, (1, D), FP)
    y_out = nc.declare_dram_parameter("y", [RROWS, D], FP, isOutput=True)
    dbg = {}
    if DEBUG:
        dbg["qT"] = nc.declare_dram_parameter("dbg_qT", [128, 8 * RROWS], FP, isOutput=True)
        dbg["sums"] = nc.declare_dram_parameter("dbg_sums", [128, 128], FP, isOutput=True)
        dbg["relq"] = nc.declare_dram_parameter("dbg_relq", [128, 64 * 512], FP, isOutput=True)
        dbg["AO"] = nc.declare_dram_parameter("dbg_AO", [128, 64 * DH], FP, isOutput=True)
        dbg["x1"] = nc.declare_dram_parameter("dbg_x1", [RROWS, D], FP, isOutput=True)
        dbg["CG"] = nc.declare_dram_parameter("dbg_CG", [128, 64 * 128], FP, isOutput=True)

    kv_loc, kv_full = {}, {}
    for s in ("1", "2"):
        kv_loc[s] = nc.dram_tensor("kv_loc" + s, [1, LOCN], BF)
        kv_full[s] = nc.dram_tensor("kv_full" + s, [NC, LOCN], BF,
                                    addr_space="Shared")
    x1_dram = nc.dram_tensor("x1_rows", [RROWS, D], FP)
    x2_dram = nc.dram_tensor("x2_rows", [RROWS, D], FP)

    groups = [list(range(NC))]

    with tile.TileContext(nc) as tc, contextlib.ExitStack() as ctx:
        const = ctx.enter_context(tc.tile_pool(name="const", bufs=1))
        perm = ctx.enter_context(tc.tile_pool(name="perm", bufs=1))
        wsl = ctx.enter_context(tc.tile_pool(name="wsl", bufs=3))
        stream = ctx.enter_context(tc.tile_pool(name="stream", bufs=3))
        attp = ctx.enter_context(tc.tile_pool(name="attp", bufs=3))
        kap = ctx.enter_context(tc.tile_pool(name="kap", bufs=2))
        relev = ctx.enter_context(tc.tile_pool(name="relev", bufs=6))
        rows = ctx.enter_context(tc.tile_pool(name="rows", bufs=2))
        h1p = ctx.enter_context(tc.tile_pool(name="h1p", bufs=4))
        lnp = ctx.enter_context(tc.tile_pool(name="lnp", bufs=1))
        stats = ctx.enter_context(tc.tile_pool(name="stats", bufs=4))
        pbig = ctx.enter_context(tc.tile_pool(name="pbig", bufs=4, space="PSUM"))
        psm = ctx.enter_context(tc.tile_pool(name="psm", bufs=4, space="PSUM"))

        ident = const.tile([128, 128], BF)
        make_identity(nc, ident[:])
        epst = const.tile([128, 1], FP)
        nc.vector.memset(epst[:], EPS)
        sums2 = const.tile([128, 128], FP, tag="sums")
        recip = const.tile([128, 64], FP, tag="recip")

        relkT, rbT = {}, {}
        for s in ("1", "2"):
            t = const.tile([2 * DH, NREL], BF, tag="relkT" + s)
            nc.sync.dma_start(out=t[:], in_=w_in["relkT" + s][:, :])
            relkT[s] = t
            t = const.tile([NREL, H], FP, tag="rbT" + s)
            nc.sync.dma_start(out=t[:], in_=w_in["rbT" + s][:, :])
            rbT[s] = t
        bias = {}
        for nm, src in bcol_in.items():
            t = const.tile(list(src.shape), FP, tag=nm)
            nc.sync.dma_start(out=t[:], in_=src[:, :])
            bias[nm] = t

        xT = perm.tile([128, 8, RROWS], BF, tag="xT")
        nc.sync.dma_start(
            out=xT[:], in_=xT_in[:, :].rearrange("(t p) j -> p t j", p=128))
        encT = perm.tile([128, 8, RROWS], BF, tag="enc_aot")
        nc.sync.dma_start(
            out=encT[:], in_=encT_in[:, :].rearrange("(t p) j -> p t j", p=128))

        def dump(name, ap, cols):
            if not DEBUG:
                return
            CH = 512
            for c0 in range(0, cols, CH):
                w = min(CH, cols - c0)
                t = stream.tile([ap.shape[0], CH], FP, tag="dump")
                nc.vector.tensor_copy(out=t[:, 0:w], in_=ap[:, c0:c0 + w])
                nc.sync.dma_start(out=dbg[name][0:ap.shape[0], c0:c0 + w],
                                  in_=t[:, 0:w])

        def load_w(dram):
            """Full [D, n] weight into two [128, 4, n] slab tiles."""
            n = dram.shape[1]
            out = []
            for half in range(2):
                t = wsl.tile([128, 4, n], BF, tag="wslab")
                nc.sync.dma_start(
                    out=t[:],
                    in_=dram[:, :].rearrange("(t p) j -> p t j", p=128)[
                        :, half * 4:(half + 1) * 4, :])
                out.append(t)
            return out

        def proj_T(wname, rhs_sb, out_sb, scale, bias_t):
            """out_sb[:, hp, :] = scale * (w^T @ rhs) + bias   (per-partition
            bias column per head-pair tile)."""
            slabs = load_w(w_in[wname])
            for hp in range(8):
                ps = pbig.tile([128, RROWS], FP, tag="pbig")
                for kt in range(8):
                    nc.tensor.matmul(
                        ps[:], slabs[kt // 4][:, kt % 4, hp * 128:(hp + 1) * 128],
                        rhs_sb[:, kt, :], start=(kt == 0), stop=(kt == 7))
                bt = bias_t[:, hp:hp + 1] if bias_t is not None else 0.0
                nc.scalar.activation(out=out_sb[:, hp, :], in_=ps[:],
                                     func=AF.Identity, bias=bt, scale=scale)

        # ============ phase A: K/V projections + allgather ==================
        kloc = {
            s: kv_loc[s][:, 0:KSEC].rearrange(
                "o (h d b k) -> o h d b k", h=H, d=DH, b=B)
            for s in ("1", "2")
        }
        vloc = {
            s: kv_loc[s][:, KSEC:].rearrange(
                "o (b h k d) -> o b h k d", b=B, h=H, k=128)
            for s in ("1", "2")
        }

        def kv_phase(s, rhs_sb):
            slabs = load_w(w_in["wkT" + s])
            for hp in range(8):
                ps = pbig.tile([128, RROWS], FP, tag="pbig")
                for kt in range(8):
                    nc.tensor.matmul(
                        ps[:], slabs[kt // 4][:, kt % 4, hp * 128:(hp + 1) * 128],
                        rhs_sb[:, kt, :], start=(kt == 0), stop=(kt == 7))
                ev = stream.tile([128, RROWS], BF, tag="kvev")
                nc.scalar.activation(out=ev[:], in_=ps[:], func=AF.Identity,
                                     bias=bias["bk%ss" % s][:, hp:hp + 1],
                                     scale=1.0)
                evv = ev[:].rearrange("p (b k) -> p b k", b=B)
                for hb in range(2):
                    nc.sync.dma_start(
                        out=kloc[s][0, 2 * hp + hb],
                        in_=evv[hb * 64:(hb + 1) * 64, :, :])
            slabs = load_w(w_in["wvT" + s])
            for b in range(B):
                for n in range(2):
                    ps = pbig.tile([128, 512], FP, tag="pbig")
                    for kt in range(8):
                        nc.tensor.matmul(
                            ps[:], rhs_sb[:, kt, b * 128:(b + 1) * 128],
                            slabs[kt // 4][:, kt % 4, n * 512:(n + 1) * 512],
                            start=(kt == 0), stop=(kt == 7))
                    ev = stream.tile([128, 512], BF, tag="kvev")
                    nc.scalar.copy(ev[:], ps[:])
                    evv = ev[:].rearrange("p (h d) -> p h d", d=DH)
                    for hh in range(8):
                        nc.sync.dma_start(
                            out=vloc[s][0, b, n * 8 + hh], in_=evv[:, hh, :])
            nc.gpsimd.collective_compute(
                "AllGather", OP.bypass, replica_groups=groups,
                ins=[kv_loc[s][:, :]], outs=[kv_full[s][:, :]])

        kv_phase("1", xT)
        kv_phase("2", encT)

        # ============ Q1 projection ========================================
        qT = perm.tile([128, 8, RROWS], BF, tag="qT")
        proj_T("wqT1", xT, qT, 0.125, bias["bq1s"])

        # ============ C / CG ===============================================
        CG = perm.tile([128, 64, 128], F8, tag="cg")

        def build_C_CG(s, qT_sb):
            """P = rel_k @ q^T per head; write (P + rel_b) straight into the
            block-diagonal CG operand: CG[qq*KR+r, g, qq*64 + b*16 + h] =
            P[r, b, 2g+qq] + rel_b[r, h]; mask rows get MASKVAL via memset."""
            nc.gpsimd.memset(CG[:], 0.0)
            nc.sync.dma_start(out=CG[NREL:NREL + 1, :, 0:64], in_=mrow_in[:, :])
            nc.sync.dma_start(out=CG[64 + NREL:64 + NREL + 1, :, 64:128],
                              in_=mrow_in[:, :])
            cgv = CG[:].rearrange("k g (q2 b j) -> k q2 b j g", q2=2, b=B)
            for h in range(H):
                hp, hb = h // 2, h % 2
                ps = psm.tile([NREL, RROWS], FP, tag="ps")
                nc.tensor.matmul(
                    ps[:], relkT[s][hb * 64:(hb + 1) * 64, :],
                    qT_sb[hb * 64:(hb + 1) * 64, hp, :],
                    start=True, stop=True)
                psv = ps[:].rearrange("r (b g q2) -> r q2 b g", b=B, q2=2)
                for qq in range(2):
                    nc.scalar.activation(
                        out=cgv[qq * 64:qq * 64 + NREL, qq, :, h, :],
                        in_=psv[:, qq, :, :], func=AF.Identity,
                        bias=rbT[s][:, h:h + 1], scale=1.0)

        build_C_CG("1", qT)
        dump("qT", qT[:].rearrange("p a b -> p (a b)"), 8 * RROWS)
        dump("CG", CG[:].rearrange("p a b -> p (a b)"), 64 * 128)

        # ============ attention ============================================
        def attn_block(s, qT_sb, aot):
            relq = perm.tile([128, 64, 512], F8, tag="big8")
            AO = perm.tile([128, 64, DH], BF, tag="c_ao")
            ks = kv_full[s][:, 0:KSEC].rearrange(
                "c (h d b k) -> c h d b k", h=H, d=DH, b=B)
            vs = kv_full[s][:, KSEC:].rearrange(
                "c (b h k d) -> c b h k d", b=B, h=H, k=128)
            for khalf in range(2):
                for g in range(64):
                    oh = stream.tile([128, 512], F8, tag="oh")
                    nc.sync.dma_start(
                        out=oh[:],
                        in_=w_in["oht" + s][g, :, khalf * 512:(khalf + 1) * 512])
                    ps = pbig.tile([128, 512], FP, tag="pbig")
                    nc.tensor.matmul(ps[:], CG[:, g, :], oh[:],
                                     start=True, stop=True)
                    ev = relev.tile([128, 512], F8, tag="relev")
                    if g % 2 == 0:
                        nc.vector.tensor_copy(out=ev[:], in_=ps[:])
                    else:
                        nc.scalar.copy(ev[:], ps[:])
                    for qq in range(2):
                        nc.sync.dma_start(
                            out=relq[2 * g + qq:2 * g + qq + 1, :, :],
                            in_=ev[qq * 64:(qq + 1) * 64, :])
                for h in range(H):
                    hp, hb = h // 2, h % 2
                    kt_ = kap.tile([128, 4, RROWS], BF, tag="kasm")
                    nc.sync.dma_start(
                        out=kt_[hb * 64:(hb + 1) * 64, :, :],
                        in_=ks[khalf * 4:(khalf + 1) * 4, h, :, :, :].rearrange(
                            "a p b k -> p a (b k)"))
                    for b in range(B):
                        bh = h * 4 + b
                        ps = pbig.tile([128, 512], FP, tag="pbig")
                        nc.tensor.matmul(
                            ps[:],
                            qT_sb[hb * 64:(hb + 1) * 64, hp, b * 128:(b + 1) * 128],
                            kt_[hb * 64:(hb + 1) * 64, :, b * 128:(b + 1) * 128],
                            start=True, stop=True)
                        lg = attp.tile([128, 512], BF, tag="lg")
                        nc.vector.tensor_add(lg[:], ps[:], relq[:, bh, :])
                        at = attp.tile([128, 512], BF, tag="at")
                        col = bh * 2 + khalf
                        nc.scalar.activation(
                            out=at[:], in_=lg[:], func=AF.Exp,
                            accum_out=sums2[:, col:col + 1])
                        att = attp.tile([128, 4, 128], BF, tag="att")
                        nc.sync.dma_start(out=att[:], in_=at[:], transpose=True)
                        vt = attp.tile([128, 4, DH], BF, tag="vasm")
                        nc.sync.dma_start(
                            out=vt[:],
                            in_=vs[khalf * 4:(khalf + 1) * 4, b, h, :, :].rearrange(
                                "a p d -> p a d"))
                        pa = psm.tile([128, DH], FP, tag="ps")
                        for m in range(4):
                            nc.tensor.matmul(pa[:], att[:, m, :], vt[:, m, :],
                                             start=(m == 0), stop=(m == 3))
                        bm = b * H + h
                        if khalf == 0:
                            nc.vector.tensor_copy(out=AO[:, bm, :], in_=pa[:])
                        else:
                            nc.vector.tensor_add(AO[:, bm, :], pa[:], AO[:, bm, :])
            sv = sums2[:].rearrange("p (a c) -> p a c", c=2)
            nc.vector.tensor_add(recip[:], sv[:, :, 0], sv[:, :, 1])
            nc.vector.reciprocal(out=recip[:], in_=recip[:])
            for b in range(B):
                for h in range(H):
                    nc.scalar.activation(
                        out=AO[:, b * H + h, :], in_=AO[:, b * H + h, :],
                        func=AF.Copy, bias=0.0,
                        scale=recip[:, h * 4 + b:h * 4 + b + 1])
            if s == "1":
                dump("AO", AO[:].rearrange("p a b -> p (a b)"), 64 * DH)
            aov = AO[:].rearrange("p (b j) d -> p b (j d)", b=B)
            for b in range(B):
                for kt in range(8):
                    pt = psm.tile([128, 128], BF, tag="ps")
                    nc.tensor.transpose(
                        pt[:], aov[:, b, kt * 128:(kt + 1) * 128], ident[:])
                    nc.vector.tensor_copy(
                        out=aot[:, kt, b * 128:(b + 1) * 128], in_=pt[:])

        AOT1 = perm.tile([128, 8, RROWS], BF, tag="enc_aot")
        attn_block("1", qT, AOT1)
        if DEBUG:
            nc.sync.dma_start(out=dbg["sums"][:, :], in_=sums2[:])

        # ============ out-proj + residual + LN =============================
        def load_lngb(s):
            g = lnp.tile([128, D], FP, tag="lng")
            bb = lnp.tile([128, D], FP, tag="lnb")
            for t, name in ((g, "g" + s), (bb, "b" + s)):
                src = ln_in[name][:, :]
                nc.sync.dma_start(
                    out=t[:],
                    in_=bass.AP(tensor=src.tensor, offset=src.offset,
                                ap=[[0, 128]] + list(src.ap[1:])))
            return g, bb

        def layer_norm_inplace(h1, g_t, b_t):
            st = stats.tile([128, 2, 6], FP, tag="bnst")
            mv = stats.tile([128, 2], FP, tag="bnmv")
            hv = h1[:].rearrange("p (a c) -> p a c", a=2)
            for sg in range(2):
                nc.vector.bn_stats(out=st[:, sg, :], in_=hv[:, sg, :])
            nc.vector.bn_aggr(out=mv[:], in_=st[:])
            rstd = stats.tile([128, 1], FP, tag="rstd")
            nc.scalar.activation(out=rstd[:], in_=mv[:, 1:2], func=AF.Sqrt,
                                 bias=epst[:], scale=1.0)
            nc.vector.reciprocal(out=rstd[:], in_=rstd[:])
            negms = stats.tile([128, 1], FP, tag="negms")
            nc.vector.scalar_tensor_tensor(
                out=negms[:], in0=mv[:, 0:1], scalar=-1.0, in1=rstd[:],
                op0=OP.mult, op1=OP.mult)
            nc.scalar.activation(out=h1[:], in_=h1[:], func=AF.Identity,
                                 bias=negms[:], scale=rstd[:])
            nc.vector.tensor_mul(h1[:], h1[:], g_t[:])
            nc.vector.tensor_add(h1[:], h1[:], b_t[:])

        def out_proj_ln(s, aot, res_ap_fn, ln_s, out_dram):
            g_t, b_t = load_lngb(ln_s)
            slabs = load_w(w_in["woT" + s])
            for b in range(B):
                h1 = h1p.tile([128, D], FP, tag="h1")
                res = res_ap_fn(b)
                for n in range(2):
                    ps = pbig.tile([128, 512], FP, tag="pbig")
                    for kt in range(8):
                        nc.tensor.matmul(
                            ps[:],
                            aot[:, kt, b * 128:(b + 1) * 128],
                            slabs[kt // 4][:, kt % 4, n * 512:(n + 1) * 512],
                            start=(kt == 0), stop=(kt == 7))
                    nc.vector.tensor_add(
                        h1[:, n * 512:(n + 1) * 512], ps[:],
                        res[:, n * 512:(n + 1) * 512])
                layer_norm_inplace(h1, g_t, b_t)
                nc.sync.dma_start(
                    out=out_dram[b * 128:(b + 1) * 128, :], in_=h1[:])

        def load_res_rows(dram):
            def fn(b):
                t = rows.tile([128, D], FP, tag="resrow")
                nc.sync.dma_start(out=t[:], in_=dram[b * 128:(b + 1) * 128, :])
                return t[:]
            return fn

        out_proj_ln("1", AOT1, load_res_rows(xres_in), "1", x1_dram)
        if DEBUG:
            for b in range(B):
                t = rows.tile([128, D], FP, tag="resrow")
                nc.sync.dma_start(out=t[:], in_=x1_dram[b * 128:(b + 1) * 128, :])
                nc.sync.dma_start(out=dbg["x1"][b * 128:(b + 1) * 128, :], in_=t[:])

        def rows_to_T(src_dram, dstT):
            for b in range(B):
                xb = rows.tile([128, D], BF, tag="xb16")
                nc.gpsimd.dma_start(
                    out=xb[:], in_=src_dram[b * 128:(b + 1) * 128, :])
                for kt in range(8):
                    pt = psm.tile([128, 128], BF, tag="ps")
                    nc.tensor.transpose(
                        pt[:], xb[:, kt * 128:(kt + 1) * 128], ident[:])
                    nc.vector.tensor_copy(
                        out=dstT[:, kt, b * 128:(b + 1) * 128], in_=pt[:])

        x1T = perm.tile([128, 8, RROWS], BF, tag="xT")
        rows_to_T(x1_dram, x1T)

        # ============ block 2 ==============================================
        qT2 = perm.tile([128, 8, RROWS], BF, tag="qT")
        proj_T("wqT2", x1T, qT2, 0.125, bias["bq2s"])
        build_C_CG("2", qT2)
        AOT2 = perm.tile([128, 8, RROWS], BF, tag="enc_aot")
        attn_block("2", qT2, AOT2)
        out_proj_ln("2", AOT2, load_res_rows(x1_dram), "2", x2_dram)
        x2T = perm.tile([128, 8, RROWS], BF, tag="xT")
        rows_to_T(x2_dram, x2T)

        # ============ FFN ==================================================
        hT = perm.tile([128, FF // 128, RROWS], BF, tag="hT")
        for mt in range(FF // 128):
            w1t = stream.tile([128, 8, 128], BF, tag="w1t")
            nc.sync.dma_start(
                out=w1t[:],
                in_=w1T_in[:, mt * 128:(mt + 1) * 128].rearrange(
                    "(t p) j -> p t j", p=128))
            ps = pbig.tile([128, RROWS], FP, tag="pbig")
            for kt in range(8):
                nc.tensor.matmul(ps[:], w1t[:, kt, :], x2T[:, kt, :],
                                 start=(kt == 0), stop=(kt == 7))
            nc.scalar.activation(
                out=hT[:, mt, :], in_=ps[:], func=AF.Relu,
                bias=bias["b1s"][:, mt:mt + 1], scale=1.0)

        g_t, b_t = load_lngb("3")
        w2v = w2T_in[:, :].rearrange("(t p) j -> p t j", p=128)
        resfn = load_res_rows(x2_dram)
        for b in range(B):
            h1 = h1p.tile([128, D], FP, tag="h1")
            res = resfn(b)
            ps0 = pbig.tile([128, 512], FP, tag="pbig")
            ps1 = pbig.tile([128, 512], FP, tag="pbig")
            for kt in range(FF // 128):
                w2t = stream.tile([128, D], BF, tag="w2t")
                nc.sync.dma_start(out=w2t[:], in_=w2v[:, kt, :])
                lhs = hT[:, kt, b * 128:(b + 1) * 128]
                nc.tensor.matmul(ps0[:], lhs, w2t[:, 0:512],
                                 start=(kt == 0), stop=(kt == FF // 128 - 1))
                nc.tensor.matmul(ps1[:], lhs, w2t[:, 512:1024],
                                 start=(kt == 0), stop=(kt == FF // 128 - 1))
            nc.vector.tensor_add(h1[:, 0:512], ps0[:], res[:, 0:512])
            nc.vector.tensor_add(h1[:, 512:1024], ps1[:], res[:, 512:1024])
            layer_norm_inplace(h1, g_t, b_t)
            nc.sync.dma_start(out=y_out[b * 128:(b + 1) * 128, :], in_=h1[:])

    nc.compile()
    return nc


# ---------------------------------------------------------------------------
# host side
# ---------------------------------------------------------------------------

def _stage_inputs(inputs):
    bf16 = ml_dtypes.bfloat16
    f8 = ml_dtypes.float8_e4m3
    f = {k: np.asarray(v) for k, v in inputs.items()}
    x = f["x"].astype(np.float32)
    enc = f["enc_output"].astype(np.float32)
    ids1 = f["dec_relative_ids1"].astype(np.int64)
    ids2 = f["dec2enc_relative_ids"].astype(np.int64)
    m1 = f["trg_mask"].astype(bool).reshape(L, L)
    m2 = f["src_mask"].astype(bool).reshape(L, L)

    def W(n):
        return f[n].astype(np.float32)

    c1 = W("wo1") @ W("bv1") + W("bo1")
    c2 = W("wo2") @ W("bv2") + W("bo2")
    bq2p = W("bq2") - W("wq2") @ c2
    b1p = W("b1") - W("w1") @ W("b2")

    shared = {
        "mrow": np.full((1, 64 * 64), MASKVAL, dtype=np.float32).astype(f8),
        "w1T": np.ascontiguousarray(W("w1").T).astype(bf16),
        "w2T": np.ascontiguousarray(W("w2").T).astype(bf16),
        "b1s": np.ascontiguousarray(b1p.reshape(FF // 128, 128).T).astype(np.float32),
        "bq1s": np.ascontiguousarray((0.125 * W("bq1")).reshape(8, 128).T).astype(np.float32),
        "bk1s": np.ascontiguousarray(W("bk1").reshape(8, 128).T).astype(np.float32),
        "bq2s": np.ascontiguousarray((0.125 * bq2p).reshape(8, 128).T).astype(np.float32),
        "bk2s": np.ascontiguousarray(W("bk2").reshape(8, 128).T).astype(np.float32),
        "lng1": W("g1").reshape(1, D).astype(np.float32),
        "lnb1": (W("be1") + c2).reshape(1, D).astype(np.float32),
        "lng2": W("g2").reshape(1, D).astype(np.float32),
        "lnb2": (W("be2") + W("b2")).reshape(1, D).astype(np.float32),
        "lng3": W("g3").reshape(1, D).astype(np.float32),
        "lnb3": W("be3").reshape(1, D).astype(np.float32),
    }
    for s in ("1", "2"):
        for nm in ("wq", "wk", "wv", "wo"):
            shared[nm + "T" + s] = np.ascontiguousarray(W(nm + s).T).astype(bf16)
        rkt = np.ascontiguousarray(W("rel_k" + s).T)
        shared["relkT" + s] = np.concatenate([rkt, rkt], axis=0).astype(bf16)
        shared["rbT" + s] = np.ascontiguousarray(0.125 * W("rel_b" + s)).astype(np.float32)

    def build_oht(ids_c, mask_c):
        g2 = ids_c.reshape(QC // 2, 2, L)
        mk = mask_c.reshape(QC // 2, 2, L)
        oh = np.zeros((QC // 2, 2, 64, L), dtype=np.float32)
        r = np.arange(NREL)
        oh[:, :, :NREL, :] = g2[:, :, None, :] == r[None, None, :, None]
        oh[:, :, NREL, :] = ~mk
        return np.ascontiguousarray(oh.reshape(QC // 2, 128, L)).astype(f8)

    maps = []
    for c in range(NC):
        qs = slice(c * QC, (c + 1) * QC)
        xc = x[:, qs, :]
        ec = enc[:, qs, :]
        m = dict(shared)
        m["xT"] = np.ascontiguousarray(
            np.moveaxis(xc, 2, 0).reshape(D, RROWS)).astype(bf16)
        m["encT"] = np.ascontiguousarray(
            np.moveaxis(ec, 2, 0).reshape(D, RROWS)).astype(bf16)
        m["xres"] = np.ascontiguousarray(
            (xc + c1).reshape(RROWS, D)).astype(np.float32)
        m["oht1"] = build_oht(ids1[qs], m1[qs])
        m["oht2"] = build_oht(ids2[qs], m2[qs])
        maps.append(m)
    return maps


def _get_built():
    global _BUILT
    if _BUILT is None:
        _BUILT = _build_program()
    return _BUILT


def kernel(**inputs):
    from concourse.bass_utils import run_bass_kernel_spmd

    prog = _get_built()
    maps = _stage_inputs(inputs)
    res = run_bass_kernel_spmd(prog, maps, list(range(NC)))
    out = np.zeros((B, L, D), dtype=np.float32)
    for c in range(NC):
        out[:, c * QC:(c + 1) * QC, :] = res.results[c]["y"].reshape(B, QC, D)
    return out
